# revision 11
# baseline (speedup 1.0000x reference)
"""MoE routing kernel for Trainium2 (8 NeuronCores, expert-parallel).

Strategy:
  - Host: compute gate (sigmoid + grouped top-k routing) in numpy, gather
    tokens per expert (sparse dispatch; top-2 of 8 experts per token).
  - Device (SPMD, core e): SwiGLU MLP with expert e's weights over the
    tokens routed to e, plus a 1/8 token-shard of the shared-expert MLP.
    Layout keeps features on SBUF partitions and streams tokens along the
    free axis, so activations feed matmuls without any on-device transpose.
  - Host: weighted scatter-add of expert outputs + shared output.
"""

import numpy as np
from contextlib import ExitStack

DIM = 768
INTER = 512
E = 8
G = 4
TOPK = 2
N_CORES = 8
P = 128
NCHUNK = 512  # tokens per PSUM tile (fp32 bank limit)


# ---------------------------------------------------------------- host gate
def _host_gate(x2, gate_weight, gate_bias):
    """Reproduces reference._gate in numpy f32. Returns (w [T,2], idx [T,2])."""
    T = x2.shape[0]
    logits = x2 @ gate_weight.T
    scores = 1.0 / (1.0 + np.exp(-logits, dtype=np.float32))
    s = scores + gate_bias
    sv = s.reshape(T, G, E // G)
    group_scores = sv.sum(-1)  # top-2 of 2 per group == sum
    gidx = np.argsort(-group_scores, axis=1, kind="stable")[:, :2]
    gmask = np.zeros((T, G), bool)
    gmask[np.arange(T)[:, None], gidx] = True
    masked = np.where(gmask[:, :, None], sv, -np.inf).reshape(T, E)
    idx = np.argsort(-masked, axis=1, kind="stable")[:, :TOPK]
    w = np.take_along_axis(scores, idx, axis=1)
    w = w / (w.sum(-1, keepdims=True) + 1e-6)
    return w.astype(np.float32), idx.astype(np.int32)


# ---------------------------------------------------------- device kernel IR
def _build_nc(cap, nsh):
    import concourse.bass as bass
    import concourse.tile as tile
    from concourse import bacc, mybir

    f32 = mybir.dt.float32
    f32r = mybir.dt.float32r
    KD = DIM // P    # 6 k-tiles over model dim
    KI = INTER // P  # 4 k-tiles over inter dim

    nc = bacc.Bacc(
        "TRN2",
        target_bir_lowering=False,
        debug=False,
        enable_asserts=False,
        num_devices=N_CORES,
    )

    xg = nc.dram_tensor("xg", [DIM, cap], f32r, kind="ExternalInput").ap()
    xs = nc.dram_tensor("xs", [DIM, nsh], f32r, kind="ExternalInput").ap()
    w1t = nc.dram_tensor("w1t", [DIM, INTER], f32r, kind="ExternalInput").ap()
    w3t = nc.dram_tensor("w3t", [DIM, INTER], f32r, kind="ExternalInput").ap()
    w2t = nc.dram_tensor("w2t", [INTER, DIM], f32r, kind="ExternalInput").ap()
    sw1t = nc.dram_tensor("sw1t", [DIM, INTER], f32r, kind="ExternalInput").ap()
    sw3t = nc.dram_tensor("sw3t", [DIM, INTER], f32r, kind="ExternalInput").ap()
    sw2t = nc.dram_tensor("sw2t", [INTER, DIM], f32r, kind="ExternalInput").ap()
    oe = nc.dram_tensor("oe", [DIM, cap], f32, kind="ExternalOutput").ap()
    oz = nc.dram_tensor("oz", [DIM, nsh], f32, kind="ExternalOutput").ap()

    with tile.TileContext(nc) as tc, ExitStack() as ctx:
        wpool = ctx.enter_context(tc.tile_pool(name="wpool", bufs=1))
        xpool = ctx.enter_context(tc.tile_pool(name="xpool", bufs=3))
        hpool = ctx.enter_context(tc.tile_pool(name="hpool", bufs=2))
        opool = ctx.enter_context(tc.tile_pool(name="opool", bufs=2))
        # p1/p3 get 3 banks each, p2 gets 2: all 8 PSUM banks in play
        ppool = ctx.enter_context(tc.tile_pool(name="ppool", bufs=3, space="PSUM"))
        ppool2 = ctx.enter_context(tc.tile_pool(name="ppool2", bufs=2, space="PSUM"))

        def load_weight(ap_, tag, eng):
            # DRAM [K, M] -> SBUF [P, K//P, M]; lhsT slices are [:, k, m*P:(m+1)*P]
            # per-k-tile DMAs spread over non-sync queues: keeps the sync queue
            # free for activation tiles and lets matmuls start on partial weights
            kt = ap_.shape[0] // P
            t = wpool.tile([P, kt, ap_.shape[1]], f32r, tag=tag)
            src = ap_.rearrange("(kt p) m -> p kt m", p=P)
            for k in range(kt):
                eng.dma_start(out=t[:, k, :], in_=src[:, k, :])
            return t

        # shared-expert weights first: the shared phase runs first below
        sw1s = load_weight(sw1t, "sw1s", nc.gpsimd)
        sw3s = load_weight(sw3t, "sw3s", nc.scalar)
        sw2s = load_weight(sw2t, "sw2s", nc.gpsimd)
        w1s = load_weight(w1t, "w1s", nc.gpsimd)
        w3s = load_weight(w3t, "w3s", nc.scalar)
        w2s = load_weight(w2t, "w2s", nc.scalar)

        def swiglu(xT, outT, a1, a3, a2, ntok, first_small=False):
            xTr = xT.rearrange("(kt p) n -> p kt n", p=P)
            oTr = outT.rearrange("(kt p) n -> p kt n", p=P)
            # chunk plan: optional small head chunk so the PE starts on a
            # cheap DMA instead of waiting for a full 512-token tile
            plan = []
            pos = 0
            if first_small and ntok > P:
                plan.append((0, P))
                pos = P
            while pos < ntok:
                n = min(NCHUNK, ntok - pos)
                plan.append((pos, n))
                pos += n
            for n0, n in plan:
                xt = xpool.tile([P, KD, NCHUNK], f32r, tag="xt")
                nc.sync.dma_start(out=xt[:, :, :n], in_=xTr[:, :, n0 : n0 + n])
                h = hpool.tile([P, KI, NCHUNK], f32r, tag="h")
                for m in range(KI):
                    p1 = ppool.tile([P, NCHUNK], f32, tag="p1")
                    for k in range(KD):
                        nc.tensor.matmul(
                            p1[:, :n],
                            a1[:, k, m * P : (m + 1) * P],
                            xt[:, k, :n],
                            start=(k == 0),
                            stop=(k == KD - 1),
                        )
                    # silu(x) = x * sigmoid(x)
                    sg = hpool.tile([P, NCHUNK], f32, tag="sg")
                    nc.scalar.activation(
                        sg[:, :n], p1[:, :n], mybir.ActivationFunctionType.Sigmoid
                    )
                    nc.vector.tensor_mul(h[:, m, :n], sg[:, :n], p1[:, :n])
                    p3 = ppool.tile([P, NCHUNK], f32, tag="p3")
                    for k in range(KD):
                        nc.tensor.matmul(
                            p3[:, :n],
                            a3[:, k, m * P : (m + 1) * P],
                            xt[:, k, :n],
                            start=(k == 0),
                            stop=(k == KD - 1),
                        )
                    nc.vector.tensor_mul(h[:, m, :n], h[:, m, :n], p3[:, :n])
                ot = opool.tile([P, KD, NCHUNK], f32, tag="ot")
                for m2 in range(KD):
                    p2 = ppool2.tile([P, NCHUNK], f32, tag="p2")
                    for k2 in range(KI):
                        nc.tensor.matmul(
                            p2[:, :n],
                            a2[:, k2, m2 * P : (m2 + 1) * P],
                            h[:, k2, :n],
                            start=(k2 == 0),
                            stop=(k2 == KI - 1),
                        )
                    nc.vector.tensor_copy(ot[:, m2, :n], p2[:, :n])
                nc.sync.dma_start(out=oTr[:, :, n0 : n0 + n], in_=ot[:, :, :n])

        # shared phase first: the routed remainder chunk (smallest) drains last
        swiglu(xs, oz, sw1s, sw3s, sw2s, nsh, first_small=True)
        swiglu(xg, oe, w1s, w3s, w2s, cap)

    nc.compile()
    return nc


# ------------------------------------------------------------------- driver
def kernel(x, gate_weight, gate_bias, w1, w2, w3, sw1, sw2, sw3):
    from concourse.bass_utils import run_bass_kernel_spmd

    B, S, D = x.shape
    x2 = np.ascontiguousarray(x.reshape(-1, D))
    T = x2.shape[0]
    nsh = T // N_CORES

    w, idx = _host_gate(x2, gate_weight, gate_bias)

    rows_per_e = [np.nonzero((idx == e).any(axis=1))[0] for e in range(E)]
    cap = max(len(r) for r in rows_per_e)
    cap = ((cap + P - 1) // P) * P

    nc = _build_nc(cap, nsh)

    x2T = np.ascontiguousarray(x2.T)  # [D, T]
    in_maps = []
    for e in range(E):
        rows = rows_per_e[e]
        xgT = np.zeros((DIM, cap), np.float32)
        xgT[:, : len(rows)] = x2T[:, rows]
        in_maps.append(
            {
                "xg": xgT,
                "xs": np.ascontiguousarray(x2T[:, e * nsh : (e + 1) * nsh]),
                "w1t": np.ascontiguousarray(w1[e].T),
                "w3t": np.ascontiguousarray(w3[e].T),
                "w2t": np.ascontiguousarray(w2[e].T),
                "sw1t": np.ascontiguousarray(sw1.T),
                "sw3t": np.ascontiguousarray(sw3.T),
                "sw2t": np.ascontiguousarray(sw2.T),
            }
        )

    r = run_bass_kernel_spmd(nc, in_maps, list(range(N_CORES)))
    globals()["LAST_RESULTS"] = r
    res = r.results

    y = np.zeros((T, D), np.float32)
    for e in range(E):
        rows = rows_per_e[e]
        cnt = len(rows)
        Oe = res[e]["oe"][:, :cnt].T  # [cnt, D]
        we = np.where(idx[rows, 0] == e, w[rows, 0], w[rows, 1]).astype(np.float32)
        y[rows] += we[:, None] * Oe
    z = np.concatenate([res[c]["oz"].T for c in range(N_CORES)], axis=0)  # [T, D]
    return (y + z).reshape(B, S, D)


# revision 13
# speedup vs baseline: 1.0393x; 1.0393x over previous
"""MoE routing kernel for Trainium2 (8 NeuronCores, expert-parallel).

Strategy:
  - Host: compute gate (sigmoid + grouped top-k routing) in numpy, gather
    tokens per expert (sparse dispatch; top-2 of 8 experts per token).
  - Device (SPMD, core e): SwiGLU MLP with expert e's weights over the
    tokens routed to e, plus a 1/8 token-shard of the shared-expert MLP.
    Layout keeps features on SBUF partitions and streams tokens along the
    free axis, so activations feed matmuls without any on-device transpose.
  - Host: weighted scatter-add of expert outputs + shared output.
"""

import numpy as np
from contextlib import ExitStack

DIM = 768
INTER = 512
E = 8
G = 4
TOPK = 2
N_CORES = 8
P = 128
NCHUNK = 512  # tokens per PSUM tile (fp32 bank limit)


# ---------------------------------------------------------------- host gate
def _host_gate(x2, gate_weight, gate_bias):
    """Reproduces reference._gate in numpy f32. Returns (w [T,2], idx [T,2])."""
    T = x2.shape[0]
    logits = x2 @ gate_weight.T
    scores = 1.0 / (1.0 + np.exp(-logits, dtype=np.float32))
    s = scores + gate_bias
    sv = s.reshape(T, G, E // G)
    group_scores = sv.sum(-1)  # top-2 of 2 per group == sum
    gidx = np.argsort(-group_scores, axis=1, kind="stable")[:, :2]
    gmask = np.zeros((T, G), bool)
    gmask[np.arange(T)[:, None], gidx] = True
    masked = np.where(gmask[:, :, None], sv, -np.inf).reshape(T, E)
    idx = np.argsort(-masked, axis=1, kind="stable")[:, :TOPK]
    w = np.take_along_axis(scores, idx, axis=1)
    w = w / (w.sum(-1, keepdims=True) + 1e-6)
    return w.astype(np.float32), idx.astype(np.int32)


# ---------------------------------------------------------- device kernel IR
def _build_nc(cap, nsh):
    import concourse.bass as bass
    import concourse.tile as tile
    from concourse import bacc, mybir

    f32 = mybir.dt.float32
    f32r = mybir.dt.float32r
    KD = DIM // P    # 6 k-tiles over model dim
    KI = INTER // P  # 4 k-tiles over inter dim

    nc = bacc.Bacc(
        "TRN2",
        target_bir_lowering=False,
        debug=False,
        enable_asserts=False,
        num_devices=N_CORES,
    )

    xg = nc.dram_tensor("xg", [DIM, cap], f32r, kind="ExternalInput").ap()
    xs = nc.dram_tensor("xs", [DIM, nsh], f32r, kind="ExternalInput").ap()
    w1t = nc.dram_tensor("w1t", [DIM, INTER], f32r, kind="ExternalInput").ap()
    w3t = nc.dram_tensor("w3t", [DIM, INTER], f32r, kind="ExternalInput").ap()
    w2t = nc.dram_tensor("w2t", [INTER, DIM], f32r, kind="ExternalInput").ap()
    sw1t = nc.dram_tensor("sw1t", [DIM, INTER], f32r, kind="ExternalInput").ap()
    sw3t = nc.dram_tensor("sw3t", [DIM, INTER], f32r, kind="ExternalInput").ap()
    sw2t = nc.dram_tensor("sw2t", [INTER, DIM], f32r, kind="ExternalInput").ap()
    oe = nc.dram_tensor("oe", [DIM, cap], f32, kind="ExternalOutput").ap()
    oz = nc.dram_tensor("oz", [DIM, nsh], f32, kind="ExternalOutput").ap()

    with tile.TileContext(nc) as tc, ExitStack() as ctx:
        wpool = ctx.enter_context(tc.tile_pool(name="wpool", bufs=1))
        xpool = ctx.enter_context(tc.tile_pool(name="xpool", bufs=3))
        hpool = ctx.enter_context(tc.tile_pool(name="hpool", bufs=2))
        opool = ctx.enter_context(tc.tile_pool(name="opool", bufs=2))
        # p1/p3 get 3 banks each, p2 gets 2: all 8 PSUM banks in play
        ppool = ctx.enter_context(tc.tile_pool(name="ppool", bufs=3, space="PSUM"))
        ppool2 = ctx.enter_context(tc.tile_pool(name="ppool2", bufs=2, space="PSUM"))

        def load_weight(ap_, tag, eng):
            # DRAM [K, M] -> SBUF [P, K//P, M]; lhsT slices are [:, k, m*P:(m+1)*P]
            # per-k-tile DMAs spread over non-sync queues: keeps the sync queue
            # free for activation tiles and lets matmuls start on partial weights
            kt = ap_.shape[0] // P
            t = wpool.tile([P, kt, ap_.shape[1]], f32r, tag=tag)
            src = ap_.rearrange("(kt p) m -> p kt m", p=P)
            for k in range(kt):
                eng.dma_start(out=t[:, k, :], in_=src[:, k, :])
            return t

        # shared-expert weights first: the shared phase runs first below
        sw1s = load_weight(sw1t, "sw1s", nc.gpsimd)
        sw3s = load_weight(sw3t, "sw3s", nc.scalar)
        sw2s = load_weight(sw2t, "sw2s", nc.gpsimd)
        w1s = load_weight(w1t, "w1s", nc.gpsimd)
        w3s = load_weight(w3t, "w3s", nc.scalar)
        w2s = load_weight(w2t, "w2s", nc.scalar)

        def swiglu(xT, outT, a1, a3, a2, ntok, split_first=False):
            xTr = xT.rearrange("(kt p) n -> p kt n", p=P)
            oTr = outT.rearrange("(kt p) n -> p kt n", p=P)
            nchunks = (ntok + NCHUNK - 1) // NCHUNK
            for c in range(nchunks):
                n0 = c * NCHUNK
                n = min(NCHUNK, ntok - n0)
                xt = xpool.tile([P, KD, NCHUNK], f32r, tag="xt")
                if split_first and c == 0:
                    # per-k-tile DMAs: first matmul starts after one 256KB
                    # transfer instead of the whole 1.5MB chunk
                    for k in range(KD):
                        nc.sync.dma_start(
                            out=xt[:, k, :n], in_=xTr[:, k, n0 : n0 + n]
                        )
                else:
                    nc.sync.dma_start(out=xt[:, :, :n], in_=xTr[:, :, n0 : n0 + n])
                h = hpool.tile([P, KI, NCHUNK], f32r, tag="h")
                for m in range(KI):
                    p1 = ppool.tile([P, NCHUNK], f32, tag="p1")
                    for k in range(KD):
                        nc.tensor.matmul(
                            p1[:, :n],
                            a1[:, k, m * P : (m + 1) * P],
                            xt[:, k, :n],
                            start=(k == 0),
                            stop=(k == KD - 1),
                        )
                    # silu(x) = x * sigmoid(x)
                    sg = hpool.tile([P, NCHUNK], f32, tag="sg")
                    nc.scalar.activation(
                        sg[:, :n], p1[:, :n], mybir.ActivationFunctionType.Sigmoid
                    )
                    nc.vector.tensor_mul(h[:, m, :n], sg[:, :n], p1[:, :n])
                    p3 = ppool.tile([P, NCHUNK], f32, tag="p3")
                    for k in range(KD):
                        nc.tensor.matmul(
                            p3[:, :n],
                            a3[:, k, m * P : (m + 1) * P],
                            xt[:, k, :n],
                            start=(k == 0),
                            stop=(k == KD - 1),
                        )
                    nc.vector.tensor_mul(h[:, m, :n], h[:, m, :n], p3[:, :n])
                ot = opool.tile([P, KD, NCHUNK], f32, tag="ot")
                for m2 in range(KD):
                    p2 = ppool2.tile([P, NCHUNK], f32, tag="p2")
                    for k2 in range(KI):
                        nc.tensor.matmul(
                            p2[:, :n],
                            a2[:, k2, m2 * P : (m2 + 1) * P],
                            h[:, k2, :n],
                            start=(k2 == 0),
                            stop=(k2 == KI - 1),
                        )
                    nc.vector.tensor_copy(ot[:, m2, :n], p2[:, :n])
                nc.sync.dma_start(out=oTr[:, :, n0 : n0 + n], in_=ot[:, :, :n])

        # shared phase first: the routed remainder chunk (smallest) drains last
        swiglu(xs, oz, sw1s, sw3s, sw2s, nsh, split_first=True)
        swiglu(xg, oe, w1s, w3s, w2s, cap)

    nc.compile()
    return nc


# ------------------------------------------------------------------- driver
def kernel(x, gate_weight, gate_bias, w1, w2, w3, sw1, sw2, sw3):
    from concourse.bass_utils import run_bass_kernel_spmd

    B, S, D = x.shape
    x2 = np.ascontiguousarray(x.reshape(-1, D))
    T = x2.shape[0]
    nsh = T // N_CORES

    w, idx = _host_gate(x2, gate_weight, gate_bias)

    rows_per_e = [np.nonzero((idx == e).any(axis=1))[0] for e in range(E)]
    cap = max(len(r) for r in rows_per_e)
    cap = ((cap + P - 1) // P) * P

    nc = _build_nc(cap, nsh)

    x2T = np.ascontiguousarray(x2.T)  # [D, T]
    in_maps = []
    for e in range(E):
        rows = rows_per_e[e]
        xgT = np.zeros((DIM, cap), np.float32)
        xgT[:, : len(rows)] = x2T[:, rows]
        in_maps.append(
            {
                "xg": xgT,
                "xs": np.ascontiguousarray(x2T[:, e * nsh : (e + 1) * nsh]),
                "w1t": np.ascontiguousarray(w1[e].T),
                "w3t": np.ascontiguousarray(w3[e].T),
                "w2t": np.ascontiguousarray(w2[e].T),
                "sw1t": np.ascontiguousarray(sw1.T),
                "sw3t": np.ascontiguousarray(sw3.T),
                "sw2t": np.ascontiguousarray(sw2.T),
            }
        )

    r = run_bass_kernel_spmd(nc, in_maps, list(range(N_CORES)))
    globals()["LAST_RESULTS"] = r
    res = r.results

    y = np.zeros((T, D), np.float32)
    for e in range(E):
        rows = rows_per_e[e]
        cnt = len(rows)
        Oe = res[e]["oe"][:, :cnt].T  # [cnt, D]
        we = np.where(idx[rows, 0] == e, w[rows, 0], w[rows, 1]).astype(np.float32)
        y[rows] += we[:, None] * Oe
    z = np.concatenate([res[c]["oz"].T for c in range(N_CORES)], axis=0)  # [T, D]
    return (y + z).reshape(B, S, D)


# revision 14
# speedup vs baseline: 1.0940x; 1.0526x over previous
"""MoE routing kernel for Trainium2 (8 NeuronCores, expert-parallel).

Strategy:
  - Host: compute gate (sigmoid + grouped top-k routing) in numpy, gather
    tokens per expert (sparse dispatch; top-2 of 8 experts per token).
  - Device (SPMD, core e): SwiGLU MLP with expert e's weights over the
    tokens routed to e, plus a 1/8 token-shard of the shared-expert MLP.
    Layout keeps features on SBUF partitions and streams tokens along the
    free axis, so activations feed matmuls without any on-device transpose.
  - Host: weighted scatter-add of expert outputs + shared output.
"""

import numpy as np
from contextlib import ExitStack

DIM = 768
INTER = 512
E = 8
G = 4
TOPK = 2
N_CORES = 8
P = 128
NCHUNK = 512  # tokens per PSUM tile (fp32 bank limit)


# ---------------------------------------------------------------- host gate
def _host_gate(x2, gate_weight, gate_bias):
    """Reproduces reference._gate in numpy f32. Returns (w [T,2], idx [T,2])."""
    T = x2.shape[0]
    logits = x2 @ gate_weight.T
    scores = 1.0 / (1.0 + np.exp(-logits, dtype=np.float32))
    s = scores + gate_bias
    sv = s.reshape(T, G, E // G)
    group_scores = sv.sum(-1)  # top-2 of 2 per group == sum
    gidx = np.argsort(-group_scores, axis=1, kind="stable")[:, :2]
    gmask = np.zeros((T, G), bool)
    gmask[np.arange(T)[:, None], gidx] = True
    masked = np.where(gmask[:, :, None], sv, -np.inf).reshape(T, E)
    idx = np.argsort(-masked, axis=1, kind="stable")[:, :TOPK]
    w = np.take_along_axis(scores, idx, axis=1)
    w = w / (w.sum(-1, keepdims=True) + 1e-6)
    return w.astype(np.float32), idx.astype(np.int32)


# ---------------------------------------------------------- device kernel IR
def _build_nc(cap, nsh):
    import concourse.bass as bass
    import concourse.tile as tile
    from concourse import bacc, mybir

    f32 = mybir.dt.float32
    f32r = mybir.dt.float32r
    KD = DIM // P    # 6 k-tiles over model dim
    KI = INTER // P  # 4 k-tiles over inter dim

    nc = bacc.Bacc(
        "TRN2",
        target_bir_lowering=False,
        debug=False,
        enable_asserts=False,
        num_devices=N_CORES,
    )

    xg = nc.dram_tensor("xg", [DIM, cap], f32r, kind="ExternalInput").ap()
    xs = nc.dram_tensor("xs", [DIM, nsh], f32r, kind="ExternalInput").ap()
    w1t = nc.dram_tensor("w1t", [DIM, INTER], f32r, kind="ExternalInput").ap()
    w3t = nc.dram_tensor("w3t", [DIM, INTER], f32r, kind="ExternalInput").ap()
    w2t = nc.dram_tensor("w2t", [INTER, DIM], f32r, kind="ExternalInput").ap()
    sw1t = nc.dram_tensor("sw1t", [DIM, INTER], f32r, kind="ExternalInput").ap()
    sw3t = nc.dram_tensor("sw3t", [DIM, INTER], f32r, kind="ExternalInput").ap()
    sw2t = nc.dram_tensor("sw2t", [INTER, DIM], f32r, kind="ExternalInput").ap()
    oe = nc.dram_tensor("oe", [DIM, cap], f32, kind="ExternalOutput").ap()
    oz = nc.dram_tensor("oz", [DIM, nsh], f32, kind="ExternalOutput").ap()

    with tile.TileContext(nc) as tc, ExitStack() as ctx:
        wpool = ctx.enter_context(tc.tile_pool(name="wpool", bufs=1))
        xpool = ctx.enter_context(tc.tile_pool(name="xpool", bufs=3))
        hpool = ctx.enter_context(tc.tile_pool(name="hpool", bufs=2))
        opool = ctx.enter_context(tc.tile_pool(name="opool", bufs=2))
        # p1/p3 get 3 banks each, p2 gets 2: all 8 PSUM banks in play
        ppool = ctx.enter_context(tc.tile_pool(name="ppool", bufs=3, space="PSUM"))
        ppool2 = ctx.enter_context(tc.tile_pool(name="ppool2", bufs=2, space="PSUM"))

        def load_weight(ap_, tag, eng):
            # DRAM [K, M] -> SBUF [P, K//P, M]; lhsT slices are [:, k, m*P:(m+1)*P]
            # per-k-tile DMAs spread over non-sync queues: keeps the sync queue
            # free for activation tiles and lets matmuls start on partial weights
            kt = ap_.shape[0] // P
            t = wpool.tile([P, kt, ap_.shape[1]], f32r, tag=tag)
            src = ap_.rearrange("(kt p) m -> p kt m", p=P)
            for k in range(kt):
                eng.dma_start(out=t[:, k, :], in_=src[:, k, :])
            return t

        # shared-expert weights first: the shared phase runs first below
        sw1s = load_weight(sw1t, "sw1s", nc.gpsimd)
        sw3s = load_weight(sw3t, "sw3s", nc.gpsimd)
        sw2s = load_weight(sw2t, "sw2s", nc.gpsimd)
        w1s = load_weight(w1t, "w1s", nc.gpsimd)
        w3s = load_weight(w3t, "w3s", nc.gpsimd)
        w2s = load_weight(w2t, "w2s", nc.gpsimd)

        def swiglu(xT, outT, a1, a3, a2, ntok, split_first=False):
            xTr = xT.rearrange("(kt p) n -> p kt n", p=P)
            oTr = outT.rearrange("(kt p) n -> p kt n", p=P)
            nchunks = (ntok + NCHUNK - 1) // NCHUNK
            for c in range(nchunks):
                n0 = c * NCHUNK
                n = min(NCHUNK, ntok - n0)
                xt = xpool.tile([P, KD, NCHUNK], f32r, tag="xt")
                if split_first and c == 0:
                    # per-k-tile DMAs: first matmul starts after one 256KB
                    # transfer instead of the whole 1.5MB chunk
                    for k in range(KD):
                        nc.sync.dma_start(
                            out=xt[:, k, :n], in_=xTr[:, k, n0 : n0 + n]
                        )
                else:
                    nc.sync.dma_start(out=xt[:, :, :n], in_=xTr[:, :, n0 : n0 + n])
                h = hpool.tile([P, KI, NCHUNK], f32r, tag="h")
                for m in range(KI):
                    p1 = ppool.tile([P, NCHUNK], f32, tag="p1")
                    for k in range(KD):
                        nc.tensor.matmul(
                            p1[:, :n],
                            a1[:, k, m * P : (m + 1) * P],
                            xt[:, k, :n],
                            start=(k == 0),
                            stop=(k == KD - 1),
                        )
                    # silu(x) = x * sigmoid(x)
                    sg = hpool.tile([P, NCHUNK], f32, tag="sg")
                    nc.scalar.activation(
                        sg[:, :n], p1[:, :n], mybir.ActivationFunctionType.Sigmoid
                    )
                    nc.vector.tensor_mul(h[:, m, :n], sg[:, :n], p1[:, :n])
                    p3 = ppool.tile([P, NCHUNK], f32, tag="p3")
                    for k in range(KD):
                        nc.tensor.matmul(
                            p3[:, :n],
                            a3[:, k, m * P : (m + 1) * P],
                            xt[:, k, :n],
                            start=(k == 0),
                            stop=(k == KD - 1),
                        )
                    nc.vector.tensor_mul(h[:, m, :n], h[:, m, :n], p3[:, :n])
                ot = opool.tile([P, KD, NCHUNK], f32, tag="ot")
                for m2 in range(KD):
                    p2 = ppool2.tile([P, NCHUNK], f32, tag="p2")
                    for k2 in range(KI):
                        nc.tensor.matmul(
                            p2[:, :n],
                            a2[:, k2, m2 * P : (m2 + 1) * P],
                            h[:, k2, :n],
                            start=(k2 == 0),
                            stop=(k2 == KI - 1),
                        )
                    nc.vector.tensor_copy(ot[:, m2, :n], p2[:, :n])
                nc.sync.dma_start(out=oTr[:, :, n0 : n0 + n], in_=ot[:, :, :n])

        # shared phase first: the routed remainder chunk (smallest) drains last
        swiglu(xs, oz, sw1s, sw3s, sw2s, nsh, split_first=True)
        swiglu(xg, oe, w1s, w3s, w2s, cap)

    nc.compile()
    return nc


# ------------------------------------------------------------------- driver
def kernel(x, gate_weight, gate_bias, w1, w2, w3, sw1, sw2, sw3):
    from concourse.bass_utils import run_bass_kernel_spmd

    B, S, D = x.shape
    x2 = np.ascontiguousarray(x.reshape(-1, D))
    T = x2.shape[0]
    nsh = T // N_CORES

    w, idx = _host_gate(x2, gate_weight, gate_bias)

    rows_per_e = [np.nonzero((idx == e).any(axis=1))[0] for e in range(E)]
    cap = max(len(r) for r in rows_per_e)
    cap = ((cap + P - 1) // P) * P

    nc = _build_nc(cap, nsh)

    x2T = np.ascontiguousarray(x2.T)  # [D, T]
    in_maps = []
    for e in range(E):
        rows = rows_per_e[e]
        xgT = np.zeros((DIM, cap), np.float32)
        xgT[:, : len(rows)] = x2T[:, rows]
        in_maps.append(
            {
                "xg": xgT,
                "xs": np.ascontiguousarray(x2T[:, e * nsh : (e + 1) * nsh]),
                "w1t": np.ascontiguousarray(w1[e].T),
                "w3t": np.ascontiguousarray(w3[e].T),
                "w2t": np.ascontiguousarray(w2[e].T),
                "sw1t": np.ascontiguousarray(sw1.T),
                "sw3t": np.ascontiguousarray(sw3.T),
                "sw2t": np.ascontiguousarray(sw2.T),
            }
        )

    r = run_bass_kernel_spmd(nc, in_maps, list(range(N_CORES)))
    globals()["LAST_RESULTS"] = r
    res = r.results

    y = np.zeros((T, D), np.float32)
    for e in range(E):
        rows = rows_per_e[e]
        cnt = len(rows)
        Oe = res[e]["oe"][:, :cnt].T  # [cnt, D]
        we = np.where(idx[rows, 0] == e, w[rows, 0], w[rows, 1]).astype(np.float32)
        y[rows] += we[:, None] * Oe
    z = np.concatenate([res[c]["oz"].T for c in range(N_CORES)], axis=0)  # [T, D]
    return (y + z).reshape(B, S, D)
